# revision 3
# baseline (speedup 1.0000x reference)
"""DispNetC correlation volume on 8 NeuronCores (Trainium2, Bass/Tile).

out[b, d, h, w] = mean_c(L[b,c,h,w] * R[b,c,h,w-d]), d in [0,40), 0 where w<d.
Data-parallel over batch (B=8 -> 1 sample per core).

Environment facts (measured): ~208us fixed dispatch overhead per execution
(additive); HBM ~87 GB/s per core; DMA cost ~= max(bytes/BW, n_descriptors *
~33ns) per queue; sub-512B write runs catastrophically slow; SWDGE (gpsimd)
~5x slower for bulk; SBUF APs cannot cross partitions in DMA (no sb2sb
transpose).

Per-core pipeline (h split into NCHUNK chunks for load/compute overlap):
1. Load L, R chunks f32 -> SBUF [c_lo(128), (c_hi, h, w)] (HWDGE, 8KB runs).
2. Per h: 2 accumulating fp32 matmuls -> PSUM Gram G[w, w'] (4 h per bank).
3. Per 4-h group: scaled copy (x 1/C) + cast -> bf16 g_all2[w, (h, 167-row)]
   where each row = 39 zeros ++ G[w, h, 0:128]. (DVE/ACT alternating.)
4. Dump g_all2 chunk -> DRAM scratch, contiguous per partition (5.3KB runs).
5. Readback the diagonal band directly: scratch viewed with row pitch
   P+1=10689 makes offset(w, h, dr) = w*10689 + h*167 + dr linear
   (dr = 39-d); reads where w < d land exactly on the 39 zero-pad columns.
   dst o_pre[h2-part, (hp, w, dr)] bf16 - 80B runs, the descriptor-bound
   step, split across queues/chunks.
6. DVE/ACT reshuffle+cast: o_pre -> o_t[h2, (d, hp, w)] f32.
7. Out DMA per chunk: [h2, (d, hp, w)] -> out[d, h, w], 1KB runs.
"""

import numpy as np

C, H, W, D = 256, 64, 128, 40
PAD = 39                  # zero columns per g row (covers w' = w-d < 0)
ROW = W + PAD             # 167
GP = H * ROW              # 10688 g_all2 free elems (write pitch)
RP = GP + 1               # 10689 readback row pitch
N_CORES = 8
NCHUNK = 2                # h chunks (2 measured best: fewer per-DMA fixed costs)
HC = H // NCHUNK          # 16 h per chunk
RB_ENGINE = "hwdge"       # "hwdge" | "gpsimd" for the band readback

_cache = {}


def _build(rb_engine=RB_ENGINE, nchunk=NCHUNK):
    import concourse.bass as bass
    import concourse.bacc as bacc
    import concourse.mybir as mybir
    from concourse.tile import TileContext

    f32 = mybir.dt.float32
    bf16 = mybir.dt.bfloat16
    hc = H // nchunk
    nc = bacc.Bacc("TRN2", target_bir_lowering=False, debug=False,
                   num_devices=N_CORES)
    l_in = nc.dram_tensor("l", [C, H, W], f32, kind="ExternalInput")
    r_in = nc.dram_tensor("r", [C, H, W], f32, kind="ExternalInput")
    out = nc.dram_tensor("out", [D, H, W], f32, kind="ExternalOutput")

    lv = l_in.ap().rearrange("(ch p) h w -> p ch h w", ch=2)
    rv = r_in.ap().rearrange("(ch p) h w -> p ch h w", ch=2)

    with TileContext(nc) as tc:
        with (
            tc.tile_pool(name="inp", bufs=2 if nchunk >= 4 else 1) as inp,
            tc.tile_pool(name="fix", bufs=1) as fix,
            tc.tile_pool(name="ps", bufs=1, space="PSUM") as psp,
            tc.tile_pool(name="dram", bufs=1, space="DRAM") as dp,
        ):
            g_all = fix.tile([128, GP], bf16, tag="gall")
            gv = g_all[:, :].rearrange("w (h k) -> w h k", k=ROW)
            o_pre = fix.tile([32, 2 * 128 * D], bf16, tag="opre")
            op4 = o_pre[:, :].rearrange("p (hp w dr) -> p hp w dr",
                                        hp=2, w=128)
            o_t = fix.tile([32, D * 2 * 128], f32, tag="ot")
            ov4 = o_t[:, :].rearrange("p (d hp w) -> p d hp w", d=D, hp=2)

            # per-chunk scratch: write pitch CGP, readback pitch CGP+1
            CGP = hc * ROW

            # zero pad columns once (covers w < d region of the band)
            nc.vector.memset(g_all[:, :], 0.0)

            out4 = out.ap().rearrange("d (h2 hp) w -> h2 d hp w", hp=2)

            for j in range(nchunk):
                h0 = j * hc
                lt = inp.tile([128, 2 * hc * W], f32, tag="lt")
                rt = inp.tile([128, 2 * hc * W], f32, tag="rt")
                lt4 = lt[:, :].rearrange("p (ch h w) -> p ch h w", ch=2, h=hc)
                rt4 = rt[:, :].rearrange("p (ch h w) -> p ch h w", ch=2, h=hc)
                nc.sync.dma_start(lt4, lv[:, :, h0:h0 + hc, :])
                nc.scalar.dma_start(rt4, rv[:, :, h0:h0 + hc, :])

                # Grams: 4 h per PSUM bank
                for g in range(hc // 4):
                    gm = psp.tile([128, 512], f32, tag=f"g{g % 4}")
                    for k in range(4):
                        hb = g * 4 + k
                        for ch in range(2):
                            nc.tensor.matmul(
                                gm[:, 128 * k:128 * (k + 1)],
                                lt4[:, ch, hb, :], rt4[:, ch, hb, :],
                                start=(ch == 0), stop=(ch == 1),
                            )
                    dst = gv[:, h0 + 4 * g:h0 + 4 * g + 4, PAD:ROW]
                    src = gm[:, :].rearrange("p (h w) -> p h w", h=4)
                    if g % 2 == 0:
                        nc.vector.tensor_scalar_mul(dst, src, 1.0 / C)
                    else:
                        nc.scalar.activation(
                            dst, src, mybir.ActivationFunctionType.Copy,
                            scale=1.0 / C)

                # dump this chunk's rows to scratch (per-partition contiguous)
                scratch = dp.tile([128 * (CGP + 1)], bf16, tag=f"sc{j}")
                sflat = scratch[:]
                wview = sflat[0:128 * CGP].rearrange("(w f) -> w f", w=128)
                eng = nc.sync if j % 2 == 0 else nc.scalar
                eng.dma_start(wview, g_all[:, h0 * ROW:(h0 + hc) * ROW])

                # band readback: o_pre[h2, hp, w, dr] <- scratch with row
                # pitch CGP+1: offset = w*(CGP+1) + hloc*167 + dr lands on
                # stored G[w, hloc, w-d] (or the zero pad when w < d).
                rview = sflat[:].rearrange("(w f) -> w f", w=128)
                rv5 = rview[:, 0:CGP].rearrange(
                    "w (h2 hp k) -> w h2 hp k", hp=2, k=ROW)
                p0 = j * (hc // 2)
                for q in range(2):
                    for hp in range(2):
                        src = rv5[64 * q:64 * q + 64, :, hp, 0:D]
                        # src axes (i, h2, dr) -> (h2, i, dr)
                        src = src.transpose([1, 0, 2])
                        dst = op4[p0:p0 + hc // 2, hp,
                                  64 * q:64 * q + 64, :]
                        if rb_engine == "gpsimd":
                            nc.gpsimd.dma_start(dst, src)
                        else:
                            eng = nc.sync if q == 0 else nc.scalar
                            eng.dma_start(dst, src)

            # reshuffle+cast to f32 (d-major), split across DVE/ACT
            # (compute engines require base partition 0 - do it once,
            # after all chunk readbacks land)
            src_all = op4[:, :, :, ::-1].transpose([0, 3, 1, 2])
            nc.vector.tensor_copy(ov4[:, 0:D // 2], src_all[:, 0:D // 2])
            nc.scalar.activation(
                ov4[:, D // 2:D], src_all[:, D // 2:D],
                mybir.ActivationFunctionType.Copy)

            # out: [h2, (d, hp, w)] -> out[d, h, w], 1KB runs, d-halves
            # split across the two HWDGE queues
            nc.sync.dma_start(out4[:, 0:D // 2], ov4[:, 0:D // 2])
            nc.scalar.dma_start(out4[:, D // 2:D], ov4[:, D // 2:D])

    nc.compile()
    return nc


def _get_program():
    if "nc" not in _cache:
        _cache["nc"] = _build()
    return _cache["nc"]


def kernel(conv3a_l: np.ndarray, conv3a_r: np.ndarray) -> np.ndarray:
    from concourse import bass_utils

    nc = _get_program()
    conv3a_l = np.ascontiguousarray(conv3a_l, dtype=np.float32)
    conv3a_r = np.ascontiguousarray(conv3a_r, dtype=np.float32)
    in_maps = [
        {"l": conv3a_l[b], "r": conv3a_r[b]} for b in range(N_CORES)
    ]
    res = bass_utils.run_bass_kernel_spmd(nc, in_maps,
                                          core_ids=list(range(N_CORES)))
    return np.stack([res.results[b]["out"] for b in range(N_CORES)], axis=0)
